# revision 24
# baseline (speedup 1.0000x reference)
"""Trainium2 Bass kernel for modulated conv1d (StyleGAN-style Conv1DMod).

Reference computation (per batch sample b):
  wm[k,c,f]  = kern[k,c,f] * coef * (style[b,c] + 1)        (modulate)
  denom[f]   = rsqrt(sum_{k,c} wm[k,c,f]^2)                 (demodulate)
  out[b,f,w] = denom[f] * sum_{k,c} wm[k,c,f] * feat[b,c,w+k-1]   (SAME conv)

Sharding: data-parallel over batch B=8 -> one sample per NeuronCore.
Demodulation is a per-(b,f) linear scale, so it is applied to the conv
*output* tiles (whose partition dim is f) instead of rescaling weights.

v8 = the measured-best v4 skeleton plus the h-interleaved contraction
mapping for the small loads:
  - contraction rounds use the partition mapping c = 2p + h (h in {0,1}).
    Adjacent channel rows then sit on the same partition, so kern loads
    as three [128, 2, 256] k-slices with one contiguous 2KB run per
    partition (~1us each vs ~3us) and style loads as [128, 2] with one
    8B run per partition (~1us vs the 3.3us 256-descriptor scatter that
    gated v4's first matmul at 15.2us). Features use a
    partition-stride-of-2-rows access pattern (same DMA cost as before).
  - the conv runs in bf16 (fp32 PSUM accumulate): same 1 col/cycle PE
    rate as fp32r, no fp32r "producer must round" verifier constraint
  - rings: sync = h0 chunks + h1 chunks 1-3 (bulk loads); scalar =
    style, kern, h1 chunk-0 pieces (interleaved with kern), stores.
    SWDGE/gpsimd is unused: its first-byte lead time measured ~3us.
  - casts fp32->bf16: chunk 0 and chunk1-h0 on the DVE (they gate the
    stream start; DVE piece casts measured 0.43us), later chunks on the
    scalar ACT (DVE saturates if it takes everything - v5 lesson).
  - a block of dummy bf16 matmuls at the head keeps the PE busy during
    the initial DMA wait so the HAM clock-gate opens (K=8/8) before the
    real matmul stream starts
  - matmul order: first group bank-major (tracks the piecewise chunk-0
    arrivals), later groups weight-major (one LDWEIGHTS per 4 matmuls)
  - conv output is demodulated into bf16 staging tiles and stored as
    bf16 (halves store traffic + kernel tail); the last chunk's stores
    alternate between the scalar and by-then-idle sync rings; host
    upcasts to fp32
"""

import numpy as np

import concourse.bass as bass
import concourse.mybir as mybir
import concourse.tile as tile

B, C, W, K, F = 8, 256, 8192, 3, 256
COEF = 1.0 / float(np.sqrt(K * C))

P = 128
NH = C // P  # 2 contraction rounds per k-tap (c = 2p + h)
FT = F // P  # 2 output-partition tiles
WCHUNK = 2048  # feature chunk width
NJ = W // WCHUNK  # 4 chunks
WTILE = 512  # matmul moving-operand width (psum bank limit)
NI = WCHUNK // WTILE  # 4 w-tiles per chunk
XCOLS = WCHUNK + 2  # chunk + 1-col halo each side

N_WARM = 30  # dummy PE-warmup matmuls (N=256 each). The HAM clock gate
# opens (K=8/8) ~3.4us in; the remainder bridges the PE to the arrival
# of the first real operands (~13us: ring-serialized head DMAs) so the
# PE never idles long enough for the gate to re-close.

MAX_WAITS = 1  # walrus codegen in this container rejects >1 sync wait per inst


def _split_sync_waits(nc, limit=MAX_WAITS):
    """Move excess sem-waits onto NoOps inserted before the offending
    instruction (same engine, program order preserved)."""
    uid = 0
    for fn in nc.m.functions:
        for bb in fn.blocks:
            insts = bb.instructions
            changed = False
            newlist = []
            for ins in insts:
                si = ins.sync_info
                if si is not None and len(si.on_wait) > limit:
                    waits = list(si.on_wait)
                    keep = waits[-limit:]
                    excess = waits[:-limit]
                    for k in range(0, len(excess), limit):
                        nop = mybir.InstNoOp(name=f"waitsplit-{uid}", ins=[], outs=[])
                        uid += 1
                        nop.engine = ins.engine
                        nop.sync_info = mybir.SyncInfo(
                            on_wait=excess[k : k + limit], on_update=[]
                        )
                        newlist.append(nop)
                    ins.sync_info = mybir.SyncInfo(
                        on_wait=keep, on_update=list(si.on_update)
                    )
                    changed = True
                newlist.append(ins)
            if changed:
                bb.instructions = newlist


def _conv1dmod_body(tc, feat, style, kern, out):
    nc = tc.nc
    f32 = mybir.dt.float32
    bf16 = mybir.dt.bfloat16

    # feature rows for round h: c = 2p + h  -> [NH, 128, W]
    fview = feat.rearrange("(p h) w -> h p w", h=NH)
    # kern k-slice: [K, 128, NH, F], one contiguous 2KB run per partition
    kvw = kern.rearrange("k (p h) f -> k p h f", h=NH)

    with (
        tc.tile_pool(name="xbuf", bufs=1) as xbuf,
        tc.tile_pool(name="xraw", bufs=2) as xraw_pool,
        tc.tile_pool(name="wbuf", bufs=1) as wbuf,
        tc.tile_pool(name="stage", bufs=3) as stage_pool,
        tc.tile_pool(name="psum", bufs=7, space="PSUM") as psum_pool,
        tc.tile_pool(name="dpsum", bufs=1, space="PSUM") as dpsum_pool,
    ):
        # ---- PE warmup: dense dummy matmuls while the first DMAs fly.
        # The HAM clock gate needs ~3.4us of sustained PE activity to open
        # to K=8/8; without this the first ~20 real matmuls run at 1.2 GHz.
        wz = wbuf.tile([P, 256], bf16, tag="warmz")
        nc.vector.memset(wz[:], 0.0)
        wps = dpsum_pool.tile([P, 256], f32, tag="dpsum")
        for _ in range(N_WARM):
            nc.tensor.matmul(wps[:], wz[:, :P], wz[:], start=True, stop=True)

        # ---- head DMAs. Ring completions serialize (~3.5us each early on),
        # so each early-critical transfer gets its own ring: kern as ONE
        # DMA first on the scalar ring, style on the otherwise-idle gpsimd
        # ring, the first h0 feature piece first on the sync ring.
        ksb = wbuf.tile([P, K, NH, F], f32, tag="ksb")
        nc.scalar.dma_start(ksb[:], kern.rearrange("k (p h) f -> p k h f", h=NH))
        ssty = wbuf.tile([P, NH], f32, tag="ssty")
        with nc.allow_non_contiguous_dma(reason="256-elem style vector"):
            nc.gpsimd.dma_start(ssty[:], style.rearrange("(p h) -> p h", h=NH))
        xt = [[None] * NJ for _ in range(NH)]

        def alloc_xt(h, j):
            t = xbuf.tile([P, XCOLS], bf16, tag=f"x_{h}_{j}", name=f"x_{h}_{j}")
            xt[h][j] = t
            raw = xraw_pool.tile(
                [P, XCOLS], f32, tag=f"xraw_{h}", name=f"xraw_{h}_{j}"
            )
            lo = j * WCHUNK - 1
            hi = j * WCHUNK + WCHUNK + 1
            dst_lo = 0
            if lo < 0:
                nc.vector.memset(t[:, 0:1], 0.0)
                dst_lo = 1
                lo = 0
            if hi > W:
                nc.vector.memset(t[:, XCOLS - 1 : XCOLS], 0.0)
                hi = W
            return t, raw, lo, hi, dst_lo

        def pieces(lo, hi, n):
            bounds = np.linspace(lo, hi, n + 1).astype(int)
            return list(zip(bounds[:-1], bounds[1:]))

        # chunk 0, h0: 4 pieces on the sync ring, DVE casts
        t0, raw0, lo, hi, dst_lo = alloc_xt(0, 0)
        h0_pieces = []
        for p0, p1 in pieces(lo, hi, 4):
            ncols = int(p1 - p0)
            off = dst_lo + int(p0 - lo)
            nc.sync.dma_start(raw0[:, off : off + ncols], fview[0, :, p0:p1])
            h0_pieces.append((off, ncols))

        # chunk-0 h1 pieces follow kern on the scalar ring
        t1, raw1, lo1, hi1, dst_lo1 = alloc_xt(1, 0)
        h1_pieces = pieces(lo1, hi1, 4)
        for p0, p1 in h1_pieces:
            ncols = int(p1 - p0)
            off = dst_lo1 + int(p0 - lo1)
            nc.scalar.dma_start(raw1[:, off : off + ncols], fview[1, :, p0:p1])

        # chunk-0 casts: h0 on the DVE, h1 on the scalar ACT (v4 topology)
        for off, ncols in h0_pieces:
            nc.vector.tensor_copy(t0[:, off : off + ncols], raw0[:, off : off + ncols])
        for p0, p1 in h1_pieces:
            ncols = int(p1 - p0)
            off = dst_lo1 + int(p0 - lo1)
            nc.scalar.copy(t1[:, off : off + ncols], raw1[:, off : off + ncols])

        def emit_load(j):
            """Steady-state whole-chunk loads: h0 on the sync ring with DVE
            casts, h1 on the scalar ring with ACT casts (v4 topology)."""
            for h in range(NH):
                t, raw, lo, hi, dst_lo = alloc_xt(h, j)
                span = hi - lo
                eng = nc.sync if h == 0 else nc.scalar
                eng.dma_start(raw[:, dst_lo : dst_lo + span], fview[h, :, lo:hi])
                cvt = nc.vector.tensor_copy if h == 0 else nc.scalar.copy
                cvt(t[:, dst_lo : dst_lo + span], raw[:, dst_lo : dst_lo + span])

        emit_load(1)

        # ---- modulate weights (bf16 out): wm[k][p,h,f] = ksb*coef*(1+s) ----
        s1 = wbuf.tile([P, NH], f32, tag="s1")
        nc.vector.tensor_scalar(
            s1[:], ssty[:], 1.0, COEF, mybir.AluOpType.add, mybir.AluOpType.mult
        )
        wm = []
        for k in range(K):
            wmt = wbuf.tile([P, NH, F], bf16, tag=f"wm_{k}", name=f"wm_{k}")
            for h in range(NH):
                nc.vector.tensor_scalar_mul(
                    wmt[:, h, :], ksb[:, k, h, :], s1[:, h : h + 1]
                )
            wm.append(wmt)

        def emit_mms(j, ft, h_outer=False):
            """NI psum accumulation groups for (chunk j, ft), weight-major:
            one (k,h) stationary load feeds all NI moving tiles. h_outer
            orders all h0 rounds first (first group: h1 pieces land later)."""
            pss = [
                psum_pool.tile([P, WTILE], f32, tag="psum", name=f"ps_{j}_{ft}_{i}")
                for i in range(NI)
            ]
            if h_outer:
                rounds = [(k, h) for h in range(NH) for k in range(K)]
            else:
                rounds = [(k, h) for k in range(K) for h in range(NH)]
            first_kh, last_kh = rounds[0], rounds[-1]
            seq = [(i, kh) for kh in rounds for i in range(NI)]
            for i, (k, h) in seq:
                nc.tensor.matmul(
                    pss[i][:],
                    wm[k][:, h, ft * P : (ft + 1) * P],
                    xt[h][j][:, i * WTILE + k : i * WTILE + k + WTILE],
                    start=((k, h) == first_kh),
                    stop=((k, h) == last_kh),
                    skip_group_check=True,
                )
            return pss

        def emit_copies(j, ft, pss):
            """Demodulating PSUM->SBUF bf16 copies + bf16 output stores."""
            st = stage_pool.tile([P, WCHUNK], bf16, tag="stage")
            for i, ps in enumerate(pss):
                nc.vector.tensor_scalar_mul(
                    st[:, i * WTILE : (i + 1) * WTILE], ps[:], denom[:, ft : ft + 1]
                )
            out_rows = slice(ft * P, (ft + 1) * P)
            # steady stores ride the otherwise-idle gpsimd (SWDGE) ring so
            # they never congest the load rings; the last chunk's stores are
            # finer and go to the by-then-idle sync/scalar rings to minimize
            # the end-of-kernel drain.
            npieces = 4 if j == NJ - 1 else 2
            piece = WCHUNK // npieces
            for h in range(npieces):
                out_cols = slice(j * WCHUNK + h * piece, j * WCHUNK + (h + 1) * piece)
                if j == NJ - 1:
                    eng = nc.sync if h % 2 else nc.scalar
                else:
                    eng = nc.gpsimd
                eng.dma_start(
                    out[out_rows, out_cols], st[:, h * piece : (h + 1) * piece]
                )

        # chunk-0 first matmul block goes ahead of everything else
        pss00 = emit_mms(0, 0, h_outer=True)

        # ---- demodulation scale: denom[f] = rsqrt(sum_{k,c} wm^2) ----
        # Emitted after the first conv block so the tiny demod matmuls do
        # not sit at the head of the in-order PE queue waiting on the DVE
        # square/sum chain.
        ssq = []
        for h in range(NH):
            sqs = []
            for k in range(K):
                sqt = wbuf.tile([P, F], f32, tag=f"sq_{h}_{k}", name=f"sq_{h}_{k}")
                nc.vector.tensor_mul(sqt[:], wm[k][:, h, :], wm[k][:, h, :])
                sqs.append(sqt)
            sst = wbuf.tile([P, F], f32, tag=f"ssq_{h}", name=f"ssq_{h}")
            nc.vector.tensor_add(sst[:], sqs[0][:], sqs[1][:])
            nc.vector.tensor_add(sst[:], sst[:], sqs[2][:])
            ssq.append(sst)
        ones = wbuf.tile([P, 1], f32, tag="ones")
        nc.vector.memset(ones[:], 1.0)
        dp = dpsum_pool.tile([P, FT], f32, tag="dpsum")
        for ft in range(FT):
            for h in range(NH):
                nc.tensor.matmul(
                    dp[:, ft : ft + 1],
                    ssq[h][:, ft * P : (ft + 1) * P],
                    ones[:],
                    start=(h == 0),
                    stop=(h == NH - 1),
                )
        denom = wbuf.tile([P, FT], f32, tag="denom")
        nc.scalar.activation(denom[:], dp[:], mybir.ActivationFunctionType.Sqrt)
        nc.vector.reciprocal(denom[:], denom[:])

        # ---- conv: chunk loads stay one chunk ahead of the matmul stream ----
        emit_copies(0, 0, pss00)
        emit_copies(0, 1, emit_mms(0, 1))
        for j in range(1, NJ):
            if j + 1 < NJ:
                emit_load(j + 1)
            for ft in range(FT):
                emit_copies(j, ft, emit_mms(j, ft))


def build_bass():
    nc = bass.Bass(name="conv1dmod")
    feat = nc.dram_tensor("feature", [C, W], mybir.dt.float32, kind="ExternalInput")
    style = nc.dram_tensor("style", [C], mybir.dt.float32, kind="ExternalInput")
    kern = nc.dram_tensor("kern", [K, C, F], mybir.dt.float32, kind="ExternalInput")
    out = nc.dram_tensor("out", [F, W], mybir.dt.bfloat16, kind="ExternalOutput")
    with tile.TileContext(nc) as tc:
        _conv1dmod_body(tc, feat, style, kern, out)
    _split_sync_waits(nc)
    return nc


_NC_CACHE = None


def kernel(feature, style, kernel):
    """Full-input entry point: shard over batch across 8 cores, run, gather."""
    global _NC_CACHE
    from concourse.bass_utils import run_bass_kernel_spmd

    if _NC_CACHE is None:
        _NC_CACHE = build_bass()
    nc = _NC_CACHE

    feature = np.ascontiguousarray(feature, dtype=np.float32)
    style = np.ascontiguousarray(style, dtype=np.float32)
    kernel = np.ascontiguousarray(kernel, dtype=np.float32)

    in_maps = [
        {"feature": feature[b], "style": style[b], "kern": kernel} for b in range(B)
    ]
    res = run_bass_kernel_spmd(nc, in_maps, core_ids=list(range(B)))
    return np.stack(
        [np.asarray(r["out"]).astype(np.float32) for r in res.results], axis=0
    )
